# revision 18
# baseline (speedup 1.0000x reference)
"""Trainium2 Bass kernel for nn_CGCA_branch (gnn_message_passing).

Math: every op between x and the relu is linear and commutes with the global
average pool, so the prefix (conv1, grouped conv2, mean, fc1, adjacency
matmul) collapses into ONE [17, C] matrix applied to the per-sample spatial
channel sums:  gc[n] = (adj @ fc1 @ M2 @ w1 / S) @ sum_s x[n, :, s].
The kernel is one big HBM stream (spatially reduce x) plus a tiny
relu->fc2->sigmoid tail.

x ships as fp8 e4m3 (output rel err ~4e-4 vs the 2e-2 tolerance, half the
HBM bytes of f16).  Layout packs FOUR consecutive channels per partition
line (c = 4p+q), keeping every DMA descriptor a contiguous 12544-byte DRAM
run - the size where the 16 per-core DMA engines peak (~427 GB/s measured).

The spatial reduction splits across three engines (measured per-chunk
costs):
 - PE  (~6.7us): fp8 DoubleRow matmuls with the folded weight (scaled by
   2^k into fp8 range; both stationary pair-slots hold the same weights, so
   the pair contraction sums spatial neighbours).  16 matmuls accumulate a
   sample into one [17, 392] PSUM tile, later accum-reduced to a column.
 - DVE (~8.6us): fused scalar_tensor_tensor (spatial halves add +
   accumulator reduce, 2 fp8/cycle/lane).
 - ACT (~15.3us): activation-copy with free accumulator.
The last three chunks are processed split by channel-group across engines
so no single engine serializes the post-stream drain.

Folds are batched: per-(q,slot) sums land in a zero-initialized staging
tile, PSUM-accumulator columns in another; 5 accumulating matmuls produce
all 8 gc columns at once.  Tail: relu (DVE, bf16) -> fc2 (bf16 matmuls) ->
tanh-sigmoid (ACT) -> affine (DVE) -> one output DMA.

Rings: the 8 x chunks ride the SP HWDGE ring alone, back-to-back; weights
ride the Activation ring; the output DMA rides the SP ring after the
stream.  Sharding: pure data parallel - batch 64 = 8 shards of 8 samples.
"""

import numpy as np
import ml_dtypes

import concourse.bass as bass
import concourse.bacc as bacc
from concourse import mybir
from concourse.bass_utils import run_bass_kernel_spmd
from concourse.tile import TileContext
from contextlib import ExitStack

# ---- problem constants (hardcoded per harness contract) ----
N, C, H, W = 64, 512, 56, 56
S = H * W                      # 3136 spatial positions
J, CA, G = 17, 272, 16
NCORES = 8
NL = N // NCORES               # 8 samples per core
Q = 4                          # channels packed per partition line (c = 4p+q)
NEG = -9e15
PEW, PEH = 784, 392            # DoubleRow window / psum width
NW = S // PEW                  # 4 windows per q-group
HALF = C // 2

# ---- schedule ----
# Every chunk splits by channel-group across engines; per-chunk shares
# (PE 2q=3.4us, DVE 1q=2.2us, ACT 1q=3.2us) all fit inside the ~4.5us
# chunk arrival window, so no engine ever backlogs and the PE stays at
# high pstate.  The last chunk gives PE the outer groups so its PSUM
# accumulator completes as late work drains.
SPLIT = {n: ['P', 'P', 'D', 'A'] for n in range(NL - 1)}
SPLIT[NL - 1] = ['P', 'D', 'A', 'P']

_ADJ = np.array([
    [1,1,0,0,0,0,0,0,0,0,0,0,0,0,0,0,0],[1,1,1,0,0,0,0,0,0,0,0,0,0,0,0,0,0],
    [0,1,1,0,0,0,1,0,0,0,0,0,0,0,0,0,0],[0,0,0,1,1,0,1,0,0,0,0,0,0,0,0,0,0],
    [0,0,0,1,1,1,0,0,0,0,0,0,0,0,0,0,0],[0,0,0,0,1,1,0,0,0,0,0,0,0,0,0,0,0],
    [0,0,1,1,0,0,1,1,0,0,0,0,0,0,0,0,0],[0,0,0,0,0,0,1,1,1,0,0,0,0,0,0,0,0],
    [0,0,0,0,0,0,0,1,1,0,0,1,1,0,0,0,1],[0,0,0,0,0,0,0,0,0,1,0,0,0,0,0,0,1],
    [0,0,0,0,0,0,0,0,0,0,1,1,0,0,0,0,0],[0,0,0,0,0,0,0,0,0,0,1,1,1,0,0,0,0],
    [0,0,0,0,0,0,0,0,1,0,0,1,1,0,0,0,0],[0,0,0,0,0,0,0,0,1,0,0,0,0,1,1,0,0],
    [0,0,0,0,0,0,0,0,0,0,0,0,0,1,1,1,0],[0,0,0,0,0,0,0,0,0,0,0,0,0,0,1,1,0],
    [0,0,0,0,0,0,0,0,1,1,0,0,0,0,0,0,1]], dtype=np.int32)
NZ_IDX = np.flatnonzero(_ADJ)  # 49 entries

F32 = mybir.dt.float32
F16 = mybir.dt.float16
BF16 = mybir.dt.bfloat16
F8 = mybir.dt.float8e4
_NC_CACHE = {}


def _build_nc() -> bass.Bass:
    nc = bacc.Bacc(None, enable_partition_id=False)
    x_d = nc.declare_dram_parameter("x", [NL, C, S], F8, isOutput=False)
    wq8_d = nc.declare_dram_parameter("wq8", [128, 2, Q, 32], F8,
                                      isOutput=False)
    wpack_d = nc.declare_dram_parameter("wpack", [128, Q * J + J], F32,
                                        isOutput=False)
    fc2b_d = nc.declare_dram_parameter("fc2b", [J, C], BF16, isOutput=False)
    out_d = nc.declare_dram_parameter("out", [NL, C], F32, isOutput=True)

    with TileContext(nc) as tc, ExitStack() as ctx:
        xpool = ctx.enter_context(tc.tile_pool(name="xpool", bufs=8))
        singles = ctx.enter_context(tc.tile_pool(name="singles", bufs=1))
        smalls = ctx.enter_context(tc.tile_pool(name="smalls", bufs=3))
        psum = ctx.enter_context(tc.tile_pool(name="psum", bufs=2,
                                              space="PSUM"))

        # ---- replicated weights on the Activation HWDGE ring ----
        wq8_sb = singles.tile([128, 2, Q, 32], F8)
        wpack_sb = singles.tile([128, Q * J + J], F32)
        fc2b_sb = singles.tile([J, C], BF16)
        nc.scalar.dma_start(out=wq8_sb, in_=wq8_d[:, :, :, :])
        nc.scalar.dma_start(out=wpack_sb, in_=wpack_d[:, :])
        nc.scalar.dma_start(out=fc2b_sb, in_=fc2b_d[:, :])
        wct4_v = wpack_sb[:, 0:Q * J].rearrange("p (q j) -> p q j", q=Q)
        i17_v = wpack_sb[0:J, Q * J:Q * J + J]

        # ---- staging ----
        # per-(q, slot) sums; zeroed so the batched fold matmuls are exact
        xs2 = singles.tile([128, Q, NL], F32)
        pcol = singles.tile([J, NL], F32)       # pacc-reduce columns
        nc.vector.memset(xs2[:, :, :], 0.0)
        nc.vector.memset(pcol[:, :], 0.0)
        t16 = singles.tile([128, S // 2], F16)  # DVE STT dummy out
        scrA = singles.tile([128, S], F16)      # ACT chunk dummy out
        scrP = singles.tile([J, PEH], F32)      # reduce dummy out
        gc_ps = psum.tile([J, NL], F32, tag="gc", bufs=1)

        # c = 4p + q; (q s) puts the 4 channels' spatial runs back-to-back:
        # each partition line is one contiguous 12544-byte DRAM run.
        xv = x_d[:, :, :].rearrange("n (p q) s -> n p (q s)", p=128, q=Q)

        xts, paccs = {}, {}

        def emit_dma(n):
            xt = xpool.tile([128, Q * S], F8, tag="xt")
            nc.sync.dma_start(out=xt, in_=xv[n])
            v = xt.rearrange("p (q s) -> p q s", q=Q)
            for q in range(Q):
                xts[(n, q)] = v[:, q, :]

        def pe_q(n, q, first, last):
            """4 DoubleRow matmuls for one q-group into sample n's pacc."""
            if n not in paccs:
                paccs[n] = psum.tile([J, PEH], F32, tag="pacc", bufs=3,
                                     name=f"pacc{n}")
            pacc = paccs[n]
            for w in range(NW):
                rhs = xts[(n, q)][:, w * PEW:(w + 1) * PEW].rearrange(
                    "p (o t) -> p o t", o=2)
                nc.tensor.matmul(pacc, lhsT=wq8_sb[:, :, q, 0:J], rhs=rhs,
                                 start=(first and w == 0),
                                 stop=(last and w == NW - 1),
                                 perf_mode=mybir.MatmulPerfMode.DoubleRow)

        def dve_q(n, q):
            nc.vector.scalar_tensor_tensor(
                out=t16, in0=xts[(n, q)][:, 0:S // 2], scalar=1.0,
                in1=xts[(n, q)][:, S // 2:S], op0=mybir.AluOpType.mult,
                op1=mybir.AluOpType.add, accum_out=xs2[:, q, n:n + 1])

        def act_q(n, q):
            nc.scalar.activation(
                out=scrA, in_=xts[(n, q)][:, :],
                func=mybir.ActivationFunctionType.Copy,
                accum_out=xs2[:, q, n:n + 1])

        def emit_chunk(n, engines):
            pe_qs = [q for q in range(Q) if engines[q] == 'P']
            for q in range(Q):
                if engines[q] == 'P':
                    pe_q(n, q, first=(q == pe_qs[0]), last=(q == pe_qs[-1]))
                elif engines[q] == 'D':
                    dve_q(n, q)
                else:
                    act_q(n, q)

        def emit_red(n, eng):
            if eng == 'A':
                nc.scalar.activation(
                    out=scrP, in_=paccs[n],
                    func=mybir.ActivationFunctionType.Copy,
                    accum_out=pcol[:, n:n + 1])
            else:
                nc.vector.tensor_scalar(
                    out=scrP, in0=paccs[n], scalar1=1.0, scalar2=0.0,
                    op0=mybir.AluOpType.mult, op1=mybir.AluOpType.add,
                    accum_out=pcol[:, n:n + 1])

        # ---- main schedule ----
        # reds ride the DVE queue right after each chunk's own pieces; the
        # pacc completes before the next chunk lands, so nothing blocks.
        for n in range(NL):
            emit_dma(n)
            emit_chunk(n, SPLIT[n])
            emit_red(n, 'D')

        # ---- batched fold: 5 accumulating matmuls -> all 8 gc columns ----
        for q in range(Q):
            nc.tensor.matmul(gc_ps, lhsT=wct4_v[:, q, :], rhs=xs2[:, q, :],
                             start=(q == 0), stop=False)
        nc.tensor.matmul(gc_ps, lhsT=i17_v, rhs=pcol[:, :],
                         start=False, stop=True)

        # ---- tail: out = sigmoid(relu(gc) @ fc2t) ----
        gcr = smalls.tile([J, NL], BF16, tag="gcr")
        nc.vector.tensor_scalar(out=gcr, in0=gc_ps, scalar1=1.0,
                                scalar2=0.0, op0=mybir.AluOpType.mult,
                                op1=mybir.AluOpType.max)
        res = singles.tile([NL, C], F32)
        o_ps = psum.tile([NL, C], F32, tag="o", bufs=1)
        nc.tensor.matmul(o_ps, lhsT=gcr, rhs=fc2b_sb[:, :],
                         start=True, stop=True)
        nc.scalar.activation(out=res, in_=o_ps,
                             func=mybir.ActivationFunctionType.Sigmoid)
        # out DMA on the Activation ring: no cross-engine hop after sigmoid
        nc.scalar.dma_start(out=out_d[:, :], in_=res)

    return nc


def _get_nc() -> bass.Bass:
    if "nc" not in _NC_CACHE:
        nc = _build_nc()
        nc.finalize()
        _NC_CACHE["nc"] = nc
    return _NC_CACHE["nc"]


def _prep_inputs(x, e, w1, w2, fc1_w, fc2_w):
    """Host-side shard + weight fold (layout/precision prep only; every
    x-proportional op — reading and reducing all of x — happens on device)."""
    x8 = np.asarray(x, dtype=np.float32).reshape(N, C, S).astype(
        ml_dtypes.float8_e4m3)

    # fold conv1 / grouped-conv2 / fc1 / (1/S mean) / adjacency-softmax
    # into one [J, C] matrix
    w1d = np.asarray(w1, dtype=np.float64)
    w2g = np.asarray(w2, dtype=np.float64).reshape(G, J, J)
    m2 = np.zeros((CA, CA), dtype=np.float64)
    for g in range(G):
        m2[g * J:(g + 1) * J, g * J:(g + 1) * J] = w2g[g]
    wcomb = np.asarray(fc1_w, np.float64) @ m2 @ (w1d / S)      # [J, C]

    emat = np.full((J * J,), NEG, dtype=np.float64)
    emat[NZ_IDX] = np.asarray(e, dtype=np.float64)[0]
    emat = emat.reshape(J, J)
    adj = np.exp(emat - emat.max(axis=1, keepdims=True))
    adj /= adj.sum(axis=1, keepdims=True)
    wadj = adj @ wcomb                                          # [J, C]

    # scale into fp8 range by a power of two (exact to fold back out)
    k = int(np.floor(np.log2(224.0 / np.abs(wadj).max())))
    ws = wadj * (2.0 ** k)
    wsT = ws.T.reshape(128, Q, J)            # [p, q, j] = ws[j, 4p+q]

    wpack = np.zeros((128, Q * J + J), dtype=np.float32)
    wpack[:, 0:Q * J] = wsT.reshape(128, Q * J)
    wpack[0:J, Q * J:Q * J + J] = np.eye(J, dtype=np.float32)
    wq8 = np.zeros((128, 2, Q, 32), dtype=ml_dtypes.float8_e4m3)
    ws8 = wsT.astype(np.float32).astype(ml_dtypes.float8_e4m3)
    wq8[:, 0, :, 0:J] = ws8
    wq8[:, 1, :, 0:J] = ws8

    # 2^-k folded into fc2 (power of two: exact in bf16)
    fc2b = np.ascontiguousarray(
        (np.asarray(fc2_w, np.float64).T * (2.0 ** -k))
    ).astype(ml_dtypes.bfloat16)

    in_maps = []
    for c in range(NCORES):
        in_maps.append({
            "x": np.ascontiguousarray(x8[c * NL:(c + 1) * NL]),
            "wq8": wq8,
            "wpack": wpack,
            "fc2b": fc2b,
        })
    return in_maps


def _run(inputs: dict, trace: bool = False, trace_cores=None):
    in_maps = _prep_inputs(**inputs)
    nc = _get_nc()
    res = run_bass_kernel_spmd(nc, in_maps, list(range(NCORES)), trace=trace,
                               trace_cores=trace_cores)
    out = np.concatenate([res.results[c]["out"] for c in range(NCORES)],
                         axis=0)
    return out.reshape(N, C, 1, 1).astype(np.float32), res


def kernel(**inputs) -> np.ndarray:
    out, _ = _run(inputs, trace=False)
    return out


# revision 20
# speedup vs baseline: 1.0410x; 1.0410x over previous
"""Trainium2 Bass kernel for nn_CGCA_branch (gnn_message_passing).

Math: every op between x and the relu is linear and commutes with the global
average pool, so the prefix (conv1, grouped conv2, mean, fc1, adjacency
matmul) collapses into ONE [17, C] matrix applied to the per-sample spatial
channel sums:  gc[n] = (adj @ fc1 @ M2 @ w1 / S) @ sum_s x[n, :, s].
The kernel is one big HBM stream (spatially reduce x) plus a tiny
relu->fc2->sigmoid tail.

x ships as fp8 e4m3 (output rel err ~4e-4 vs the 2e-2 tolerance, half the
HBM bytes of f16).  Layout packs FOUR consecutive channels per partition
line (c = 4p+q), keeping every DMA descriptor a contiguous 12544-byte DRAM
run - the size where the 16 per-core DMA engines peak (~427 GB/s measured).

The spatial reduction splits across three engines (measured per-chunk
costs):
 - PE  (~6.7us): fp8 DoubleRow matmuls with the folded weight (scaled by
   2^k into fp8 range; both stationary pair-slots hold the same weights, so
   the pair contraction sums spatial neighbours).  16 matmuls accumulate a
   sample into one [17, 392] PSUM tile, later accum-reduced to a column.
 - DVE (~8.6us): fused scalar_tensor_tensor (spatial halves add +
   accumulator reduce, 2 fp8/cycle/lane).
 - ACT (~15.3us): activation-copy with free accumulator.
The last three chunks are processed split by channel-group across engines
so no single engine serializes the post-stream drain.

Folds are batched: per-(q,slot) sums land in a zero-initialized staging
tile, PSUM-accumulator columns in another; 5 accumulating matmuls produce
all 8 gc columns at once.  Tail: relu (DVE, bf16) -> fc2 (bf16 matmuls) ->
tanh-sigmoid (ACT) -> affine (DVE) -> one output DMA.

Rings: the 8 x chunks ride the SP HWDGE ring alone, back-to-back; weights
ride the Activation ring; the output DMA rides the SP ring after the
stream.  Sharding: pure data parallel - batch 64 = 8 shards of 8 samples.
"""

import numpy as np
import ml_dtypes

import concourse.bass as bass
import concourse.bacc as bacc
from concourse import mybir
from concourse.bass_utils import run_bass_kernel_spmd
from concourse.tile import TileContext
from contextlib import ExitStack

# ---- problem constants (hardcoded per harness contract) ----
N, C, H, W = 64, 512, 56, 56
S = H * W                      # 3136 spatial positions
J, CA, G = 17, 272, 16
NCORES = 8
NL = N // NCORES               # 8 samples per core
Q = 4                          # channels packed per partition line (c = 4p+q)
NEG = -9e15
PEW, PEH = 784, 392            # DoubleRow window / psum width
NW = S // PEW                  # 4 windows per q-group
HALF = C // 2

# ---- schedule ----
# full chunks: sample -> engine;  split chunks: sample -> [engine per q]
FULL = {0: 'A', 1: 'D', 2: 'P', 3: 'D', 4: 'P'}
SPLIT = {5: ['P', 'P', 'D', 'D'], 6: ['P', 'D', 'A', 'P'],
         7: ['P', 'D', 'D', 'A']}
# pacc reduce engine per PE-involved sample ('A' or 'D') in emission order
REDS = [(2, 'A'), (4, 'A'), (5, 'A'), (6, 'D'), (7, 'A')]

_ADJ = np.array([
    [1,1,0,0,0,0,0,0,0,0,0,0,0,0,0,0,0],[1,1,1,0,0,0,0,0,0,0,0,0,0,0,0,0,0],
    [0,1,1,0,0,0,1,0,0,0,0,0,0,0,0,0,0],[0,0,0,1,1,0,1,0,0,0,0,0,0,0,0,0,0],
    [0,0,0,1,1,1,0,0,0,0,0,0,0,0,0,0,0],[0,0,0,0,1,1,0,0,0,0,0,0,0,0,0,0,0],
    [0,0,1,1,0,0,1,1,0,0,0,0,0,0,0,0,0],[0,0,0,0,0,0,1,1,1,0,0,0,0,0,0,0,0],
    [0,0,0,0,0,0,0,1,1,0,0,1,1,0,0,0,1],[0,0,0,0,0,0,0,0,0,1,0,0,0,0,0,0,1],
    [0,0,0,0,0,0,0,0,0,0,1,1,0,0,0,0,0],[0,0,0,0,0,0,0,0,0,0,1,1,1,0,0,0,0],
    [0,0,0,0,0,0,0,0,1,0,0,1,1,0,0,0,0],[0,0,0,0,0,0,0,0,1,0,0,0,0,1,1,0,0],
    [0,0,0,0,0,0,0,0,0,0,0,0,0,1,1,1,0],[0,0,0,0,0,0,0,0,0,0,0,0,0,0,1,1,0],
    [0,0,0,0,0,0,0,0,1,1,0,0,0,0,0,0,1]], dtype=np.int32)
NZ_IDX = np.flatnonzero(_ADJ)  # 49 entries

F32 = mybir.dt.float32
F16 = mybir.dt.float16
BF16 = mybir.dt.bfloat16
F8 = mybir.dt.float8e4
_NC_CACHE = {}


def _build_nc() -> bass.Bass:
    nc = bacc.Bacc(None, enable_partition_id=False)
    x_d = nc.declare_dram_parameter("x", [NL, C, S], F8, isOutput=False)
    wq8_d = nc.declare_dram_parameter("wq8", [128, 2, Q, 32], F8,
                                      isOutput=False)
    wpack_d = nc.declare_dram_parameter("wpack", [128, Q * J + J], F32,
                                        isOutput=False)
    fc2b_d = nc.declare_dram_parameter("fc2b", [J, C], BF16, isOutput=False)
    out_d = nc.declare_dram_parameter("out", [NL, C], F32, isOutput=True)

    with TileContext(nc) as tc, ExitStack() as ctx:
        xpool = ctx.enter_context(tc.tile_pool(name="xpool", bufs=8))
        singles = ctx.enter_context(tc.tile_pool(name="singles", bufs=1))
        smalls = ctx.enter_context(tc.tile_pool(name="smalls", bufs=3))
        psum = ctx.enter_context(tc.tile_pool(name="psum", bufs=2,
                                              space="PSUM"))

        # ---- replicated weights on the Activation HWDGE ring ----
        wq8_sb = singles.tile([128, 2, Q, 32], F8)
        wpack_sb = singles.tile([128, Q * J + J], F32)
        fc2b_sb = singles.tile([J, C], BF16)
        nc.scalar.dma_start(out=wq8_sb, in_=wq8_d[:, :, :, :])
        nc.scalar.dma_start(out=wpack_sb, in_=wpack_d[:, :])
        nc.scalar.dma_start(out=fc2b_sb, in_=fc2b_d[:, :])
        wct4_v = wpack_sb[:, 0:Q * J].rearrange("p (q j) -> p q j", q=Q)
        i17_v = wpack_sb[0:J, Q * J:Q * J + J]

        # ---- staging ----
        # per-(q, slot) sums; zeroed so the batched fold matmuls are exact
        xs2 = singles.tile([128, Q, NL], F32)
        pcol = singles.tile([J, NL], F32)       # pacc-reduce columns
        nc.vector.memset(xs2[:, :, :], 0.0)
        nc.vector.memset(pcol[:, :], 0.0)
        t16 = singles.tile([128, S // 2], F16)  # DVE STT dummy out
        scrA = singles.tile([128, S], F16)      # ACT chunk dummy out
        scrP = singles.tile([J, PEH], F32)      # reduce dummy out
        gc_ps = psum.tile([J, NL], F32, tag="gc", bufs=1)

        # c = 4p + q; (q s) puts the 4 channels' spatial runs back-to-back:
        # each partition line is one contiguous 12544-byte DRAM run.
        xv = x_d[:, :, :].rearrange("n (p q) s -> n p (q s)", p=128, q=Q)

        xts, paccs = {}, {}

        def emit_dma(n):
            xt = xpool.tile([128, Q * S], F8, tag="xt")
            nc.sync.dma_start(out=xt, in_=xv[n])
            v = xt.rearrange("p (q s) -> p q s", q=Q)
            for q in range(Q):
                xts[(n, q)] = v[:, q, :]

        def pe_q(n, q, first, last):
            """4 DoubleRow matmuls for one q-group into sample n's pacc."""
            if n not in paccs:
                paccs[n] = psum.tile([J, PEH], F32, tag="pacc", bufs=3,
                                     name=f"pacc{n}")
            pacc = paccs[n]
            for w in range(NW):
                rhs = xts[(n, q)][:, w * PEW:(w + 1) * PEW].rearrange(
                    "p (o t) -> p o t", o=2)
                nc.tensor.matmul(pacc, lhsT=wq8_sb[:, :, q, 0:J], rhs=rhs,
                                 start=(first and w == 0),
                                 stop=(last and w == NW - 1),
                                 perf_mode=mybir.MatmulPerfMode.DoubleRow)

        def dve_q(n, q):
            nc.vector.scalar_tensor_tensor(
                out=t16, in0=xts[(n, q)][:, 0:S // 2], scalar=1.0,
                in1=xts[(n, q)][:, S // 2:S], op0=mybir.AluOpType.mult,
                op1=mybir.AluOpType.add, accum_out=xs2[:, q, n:n + 1])

        def act_q(n, q):
            nc.scalar.activation(
                out=scrA, in_=xts[(n, q)][:, :],
                func=mybir.ActivationFunctionType.Copy,
                accum_out=xs2[:, q, n:n + 1])

        def emit_chunk(n, engines):
            pe_qs = [q for q in range(Q) if engines[q] == 'P']
            for q in range(Q):
                if engines[q] == 'P':
                    pe_q(n, q, first=(q == pe_qs[0]), last=(q == pe_qs[-1]))
                elif engines[q] == 'D':
                    dve_q(n, q)
                else:
                    act_q(n, q)

        def emit_red(n, eng):
            if eng == 'A':
                nc.scalar.activation(
                    out=scrP, in_=paccs[n],
                    func=mybir.ActivationFunctionType.Copy,
                    accum_out=pcol[:, n:n + 1])
            else:
                nc.vector.tensor_scalar(
                    out=scrP, in0=paccs[n], scalar1=1.0, scalar2=0.0,
                    op0=mybir.AluOpType.mult, op1=mybir.AluOpType.add,
                    accum_out=pcol[:, n:n + 1])

        # ---- main schedule ----
        for n in range(NL):
            emit_dma(n)
            if n in FULL:
                emit_chunk(n, [FULL[n]] * Q)
            else:
                emit_chunk(n, SPLIT[n])
        for n, eng in REDS:
            emit_red(n, eng)

        # ---- batched fold: 5 accumulating matmuls -> all 8 gc columns ----
        for q in range(Q):
            nc.tensor.matmul(gc_ps, lhsT=wct4_v[:, q, :], rhs=xs2[:, q, :],
                             start=(q == 0), stop=False)
        nc.tensor.matmul(gc_ps, lhsT=i17_v, rhs=pcol[:, :],
                         start=False, stop=True)

        # ---- tail: out = sigmoid(relu(gc) @ fc2t) ----
        gcr = smalls.tile([J, NL], BF16, tag="gcr")
        nc.vector.tensor_scalar(out=gcr, in0=gc_ps, scalar1=1.0,
                                scalar2=0.0, op0=mybir.AluOpType.mult,
                                op1=mybir.AluOpType.max)
        res = singles.tile([NL, C], F32)
        o_ps = psum.tile([NL, C], F32, tag="o", bufs=1)
        nc.tensor.matmul(o_ps, lhsT=gcr, rhs=fc2b_sb[:, :],
                         start=True, stop=True)
        nc.scalar.activation(out=res, in_=o_ps,
                             func=mybir.ActivationFunctionType.Sigmoid)
        # out DMA on the Activation ring: no cross-engine hop after sigmoid
        nc.scalar.dma_start(out=out_d[:, :], in_=res)

    return nc


def _get_nc() -> bass.Bass:
    if "nc" not in _NC_CACHE:
        nc = _build_nc()
        nc.finalize()
        _NC_CACHE["nc"] = nc
    return _NC_CACHE["nc"]


def _prep_inputs(x, e, w1, w2, fc1_w, fc2_w):
    """Host-side shard + weight fold (layout/precision prep only; every
    x-proportional op — reading and reducing all of x — happens on device)."""
    x8 = np.asarray(x, dtype=np.float32).reshape(N, C, S).astype(
        ml_dtypes.float8_e4m3)

    # fold conv1 / grouped-conv2 / fc1 / (1/S mean) / adjacency-softmax
    # into one [J, C] matrix
    w1d = np.asarray(w1, dtype=np.float64)
    w2g = np.asarray(w2, dtype=np.float64).reshape(G, J, J)
    m2 = np.zeros((CA, CA), dtype=np.float64)
    for g in range(G):
        m2[g * J:(g + 1) * J, g * J:(g + 1) * J] = w2g[g]
    wcomb = np.asarray(fc1_w, np.float64) @ m2 @ (w1d / S)      # [J, C]

    emat = np.full((J * J,), NEG, dtype=np.float64)
    emat[NZ_IDX] = np.asarray(e, dtype=np.float64)[0]
    emat = emat.reshape(J, J)
    adj = np.exp(emat - emat.max(axis=1, keepdims=True))
    adj /= adj.sum(axis=1, keepdims=True)
    wadj = adj @ wcomb                                          # [J, C]

    # scale into fp8 range by a power of two (exact to fold back out)
    k = int(np.floor(np.log2(224.0 / np.abs(wadj).max())))
    ws = wadj * (2.0 ** k)
    wsT = ws.T.reshape(128, Q, J)            # [p, q, j] = ws[j, 4p+q]

    wpack = np.zeros((128, Q * J + J), dtype=np.float32)
    wpack[:, 0:Q * J] = wsT.reshape(128, Q * J)
    wpack[0:J, Q * J:Q * J + J] = np.eye(J, dtype=np.float32)
    wq8 = np.zeros((128, 2, Q, 32), dtype=ml_dtypes.float8_e4m3)
    ws8 = wsT.astype(np.float32).astype(ml_dtypes.float8_e4m3)
    wq8[:, 0, :, 0:J] = ws8
    wq8[:, 1, :, 0:J] = ws8

    # 2^-k folded into fc2 (power of two: exact in bf16)
    fc2b = np.ascontiguousarray(
        (np.asarray(fc2_w, np.float64).T * (2.0 ** -k))
    ).astype(ml_dtypes.bfloat16)

    in_maps = []
    for c in range(NCORES):
        in_maps.append({
            "x": np.ascontiguousarray(x8[c * NL:(c + 1) * NL]),
            "wq8": wq8,
            "wpack": wpack,
            "fc2b": fc2b,
        })
    return in_maps


def _run(inputs: dict, trace: bool = False, trace_cores=None):
    in_maps = _prep_inputs(**inputs)
    nc = _get_nc()
    res = run_bass_kernel_spmd(nc, in_maps, list(range(NCORES)), trace=trace,
                               trace_cores=trace_cores)
    out = np.concatenate([res.results[c]["out"] for c in range(NCORES)],
                         axis=0)
    return out.reshape(N, C, 1, 1).astype(np.float32), res


def kernel(**inputs) -> np.ndarray:
    out, _ = _run(inputs, trace=False)
    return out
